# Initial kernel scaffold
#
"""Trainium2 Bass kernel for a scalar-feature GCN critic head.

Math (rank-1 collapse of the reference):
    deg  = bincount(dst) + 1
    dinv = rsqrt(deg)
    y    = state * dinv
    z[d] = sum_{e: dst[e]=d} y[src[e]]
    agg  = dinv * z + dinv^2 * state
    out  = agg * (W1[0] @ W2) + (b1 @ W2 + b2)          # (N, 1)

Sharding: nodes are partitioned across 8 NeuronCores by destination id
(graph/data parallel); edges are bucketed by destination on the host so
every segment-sum stays device-local.  Each core rebuilds the full y
table (1 MB) itself, so no collectives are needed.  On-device, the
per-edge y[src] fetch uses per-partition indirect DMAs (one SWDGE call
per partition row; each descriptor gathers one 4-byte element), and the
per-node segment sum is a fully regular degree-classed padded-CSR
tensor_reduce.  The tiny W1/W2/b1/b2 head collapses to two scalars
computed on-device with two small PE matmuls.
"""

import sys
import os
import types
import numpy as np

sys.path.insert(0, "/opt/trn_rl_repo")

N = 250_000
E = 8_000_000
H = 1024
NCORES = 8
NPC = N // NCORES            # 31250 dst nodes per core
P = 128
NODES_PER_PART = (NPC + P - 1) // P      # 245 node slots per partition
TAB_COLS = 1954              # 128*1954 = 250112 >= N (y-table layout)
NTAB = P * TAB_COLS
DUMMY = NTAB - 1             # pad gather index (y == 0 there)
GATHER_NSUB = 1              # sub-calls per partition row

_cached = {}


def _install_axon_shim():
    """Restore the missing antenv.axon_hooks NTFF hook (profiling only) and
    neutralize artifact upload. Harmless when tracing is off."""
    if "antenv.axon_hooks" not in sys.modules:
        mod = types.ModuleType("antenv.axon_hooks")
        mod._hook = None
        mod.set_axon_ntff_profile_hook = lambda h: setattr(mod, "_hook", h)
        mod.get_axon_ntff_profile_hook = lambda: mod._hook
        sys.modules["antenv.axon_hooks"] = mod
        try:
            import antenv
            antenv.axon_hooks = mod
        except Exception:
            pass
    mod = sys.modules["antenv.axon_hooks"]
    if mod._hook is None:
        try:
            from trn_agent_boot.trn_boot import _ntff_profile_via_ctypes
            mod._hook = _ntff_profile_via_ctypes("/opt/axon/libaxon_pjrt.so")
        except Exception:
            pass
    try:
        from concourse import bass_utils
        bass_utils.upload_artifacts = lambda tmpdir: tmpdir
    except Exception:
        pass


def _ceil_to(x, m):
    return (x + m - 1) // m * m


def _ranges(counts):
    counts = np.asarray(counts, dtype=np.int64)
    total = int(counts.sum())
    if total == 0:
        return np.zeros(0, dtype=np.int64)
    ends = np.cumsum(counts)
    out = np.arange(total, dtype=np.int64)
    out -= np.repeat(ends - counts, counts)
    return out


# ----------------------------------------------------------------------------
# Host-side sharding / bucketing (index-only preprocessing)
# ----------------------------------------------------------------------------
def host_prep(edge_index):
    """Bucket edges by destination core and build, for each core:
      - packed   : [128, 128*ncols] int32 gather-offset tile (one column
                   window per indirect-DMA call, packed partition-fastest)
      - slot_nodes: [128, n_slots] original node id per CSR slot (-1 = pad)
    plus the shared reduce run-list (col0, j0, nbatch, K)."""
    src = np.ascontiguousarray(edge_index[0]).astype(np.int64)
    dst = np.ascontiguousarray(edge_index[1]).astype(np.int64)

    deg_all = np.bincount(dst, minlength=N).astype(np.int64)
    order = np.argsort(dst, kind="stable")
    src_s = src[order]
    rowptr = np.zeros(N + 1, dtype=np.int64)
    np.cumsum(deg_all, out=rowptr[1:])

    n_slots = NODES_PER_PART
    per_core_nodes = []
    Kshared = np.full(n_slots, 2, dtype=np.int64)
    for k in range(NCORES):
        lo = k * NPC
        node_ids = np.arange(lo, lo + NPC)
        degs = deg_all[lo:lo + NPC]
        rank = np.argsort(-degs, kind="stable")
        slot_nodes = np.full(n_slots * P, -1, dtype=np.int64)
        slot_nodes[:NPC] = node_ids[rank]
        slot_nodes = slot_nodes.reshape(n_slots, P)          # [j, p]
        slot_deg = np.zeros((n_slots, P), dtype=np.int64)
        v = slot_nodes >= 0
        slot_deg[v] = deg_all[slot_nodes[v]]
        Kshared = np.maximum(Kshared, slot_deg.max(axis=1))
        per_core_nodes.append(slot_nodes)

    offs = np.zeros(n_slots + 1, dtype=np.int64)
    np.cumsum(Kshared, out=offs[1:])
    G = _ceil_to(int(offs[-1]), P * GATHER_NSUB)
    ncols = G // P

    runs = []
    j = 0
    while j < n_slots:
        j2 = j
        while j2 < n_slots and Kshared[j2] == Kshared[j]:
            j2 += 1
        runs.append((int(offs[j]), j, j2 - j, int(Kshared[j])))
        j = j2

    cores = []
    for k in range(NCORES):
        slot_nodes = per_core_nodes[k]
        offs_logical = np.full((P, G), DUMMY, dtype=np.int64)
        jj, pp = np.nonzero(slot_nodes >= 0)
        nn = slot_nodes[jj, pp]
        dd = deg_all[nn]
        edge_rows = np.repeat(pp, dd)
        tgt_pos = np.repeat(offs[jj], dd) + _ranges(dd)
        src_pos = np.repeat(rowptr[nn], dd) + _ranges(dd)
        offs_logical[edge_rows, tgt_pos] = src_s[src_pos]

        # call p consumes offsets partition-fastest starting at the base of
        # its column window: packed[g%128, p*ncols + g//128] = logical[p, g]
        packed = np.empty((P, P * ncols), dtype=np.int32)
        for p in range(P):
            packed[:, p * ncols:(p + 1) * ncols] = \
                offs_logical[p].reshape(ncols, P).T
        cores.append(dict(packed=packed, slot_nodes=slot_nodes.T))

    return dict(deg_all=deg_all, cores=cores, runs=runs, G=G, ncols=ncols,
                n_slots=n_slots)


# ----------------------------------------------------------------------------
# Device kernel construction
# ----------------------------------------------------------------------------
def _build_kernel(G, ncols, runs, n_slots):
    from concourse import bass, bacc, tile, mybir
    from contextlib import ExitStack

    nc = bacc.Bacc("TRN2", target_bir_lowering=False, debug=False,
                   num_devices=NCORES)
    f32 = mybir.dt.float32

    ap_state = nc.dram_tensor("state2d", (P, TAB_COLS), f32, kind="ExternalInput").ap()
    ap_deg = nc.dram_tensor("deg2d", (P, TAB_COLS), f32, kind="ExternalInput").ap()
    ap_offs = nc.dram_tensor("offs", (P, P * ncols), mybir.dt.int32, kind="ExternalInput").ap()
    ap_sperm = nc.dram_tensor("state_perm", (P, n_slots), f32, kind="ExternalInput").ap()
    ap_dperm = nc.dram_tensor("deg_perm", (P, n_slots), f32, kind="ExternalInput").ap()
    ap_w1 = nc.dram_tensor("w1r", (P, H // P), f32, kind="ExternalInput").ap()
    ap_w2 = nc.dram_tensor("w2r", (P, H // P), f32, kind="ExternalInput").ap()
    ap_b1 = nc.dram_tensor("b1r", (P, H // P), f32, kind="ExternalInput").ap()
    ap_b2 = nc.dram_tensor("b2v", (1, 1), f32, kind="ExternalInput").ap()
    ap_out = nc.dram_tensor("outp", (P, n_slots), f32, kind="ExternalOutput").ap()
    y_dram = nc.dram_tensor("y_scratch", (NTAB, 1), f32, kind="Internal").ap()

    with tile.TileContext(nc) as tc:
        with ExitStack() as ctx:
            pool = ctx.enter_context(tc.tile_pool(name="sbuf", bufs=1))
            psum = ctx.enter_context(tc.tile_pool(name="psum", bufs=1, space="PSUM"))

            t_state = pool.tile([P, TAB_COLS], f32)
            t_deg = pool.tile([P, TAB_COLS], f32)
            t_dinv = pool.tile([P, TAB_COLS], f32)
            t_y = pool.tile([P, TAB_COLS], f32)
            t_offs = pool.tile([P, P * ncols], mybir.dt.int32)
            t_m = pool.tile([P, G + 1], f32)
            t_fence = pool.tile([P, 1], f32)
            t_dummy = pool.tile([P, 1], mybir.dt.int32)
            t_z = pool.tile([P, n_slots], f32)
            t_sperm = pool.tile([P, n_slots], f32)
            t_dperm = pool.tile([P, n_slots], f32)
            t_dinvp = pool.tile([P, n_slots], f32)
            t_w1 = pool.tile([P, H // P], f32)
            t_w2 = pool.tile([P, H // P], f32)
            t_b1 = pool.tile([P, H // P], f32)
            t_b2 = pool.tile([1, 1], f32)
            t_tmp8 = pool.tile([P, H // P], f32)
            t_uv = pool.tile([P, 2], f32)
            t_ones_col = pool.tile([P, 1], f32)
            t_ones_row = pool.tile([1, P], f32)
            t_st_row = pool.tile([1, 2], f32)
            t_st_bc = pool.tile([P, 2], f32)
            t_bias0 = pool.tile([P, 1], f32)
            t_out = pool.tile([P, n_slots], f32)
            p_st = psum.tile([1, 2], f32, space="PSUM")
            p_bc = psum.tile([P, 2], f32, space="PSUM")

            s_y = nc.alloc_semaphore("s_y")
            s_fence = nc.alloc_semaphore("s_fence")

            # ---- loads -------------------------------------------------
            nc.sync.dma_start(t_state[:], ap_state[:])
            nc.sync.dma_start(t_deg[:], ap_deg[:])
            nc.sync.dma_start(t_offs[:], ap_offs[:])
            nc.sync.dma_start(t_sperm[:], ap_sperm[:])
            nc.sync.dma_start(t_dperm[:], ap_dperm[:])
            nc.sync.dma_start(t_w1[:], ap_w1[:])
            nc.sync.dma_start(t_w2[:], ap_w2[:])
            nc.sync.dma_start(t_b1[:], ap_b1[:])
            nc.sync.dma_start(t_b2[:], ap_b2[:])
            nc.gpsimd.memset(t_fence[:], 1.0)
            nc.gpsimd.memset(t_bias0[:], 0.0)
            nc.gpsimd.memset(t_ones_col[:], 1.0)
            nc.gpsimd.memset(t_ones_row[:], 1.0)

            # ---- y table: y = state * rsqrt(deg) ----------------------
            nc.scalar.activation(t_dinv[:], t_deg[:],
                                 mybir.ActivationFunctionType.Sqrt,
                                 bias=t_bias0[:])
            nc.vector.reciprocal(t_dinv[:], t_dinv[:])
            nc.vector.tensor_mul(t_y[:], t_state[:], t_dinv[:])
            nc.gpsimd.dma_start(
                y_dram[:].rearrange("(p c) d -> p (c d)", p=P), t_y[:]
            ).then_inc(s_y, 16)

            # ensure offsets resident before descriptor generation
            nc.gpsimd.tensor_copy(t_dummy[:], t_offs[:, :1])

            # ---- gather: 128 per-partition indirect DMAs ---------------
            nch = 1
            with nc.named_scope("gather"):
                for p in range(P):
                    inst = nc.gpsimd.indirect_dma_start(
                        out=t_m[p:p + 1, :G].rearrange("p (g d) -> p g d", d=1),
                        out_offset=None,
                        in_=y_dram[:],
                        in_offset=bass.IndirectOffsetOnAxis(
                            ap=t_offs[:, p * ncols:(p + 1) * ncols], axis=0),
                    )
                    if p == 0:
                        inst._wait_ge(s_y, 16)
                # fence: same SWDGE ring -> FIFO drain; completion sem below
                nc.gpsimd.dma_start(t_m[:, G:G + 1], t_fence[:]) \
                    .then_inc(s_fence, 16)

            # ---- degree-classed segment reduce ------------------------
            with nc.named_scope("reduce"):
                first = True
                for (col0, j0, nb, kk) in runs:
                    inst = nc.vector.tensor_reduce(
                        out=t_z[:, j0:j0 + nb],
                        in_=t_m[:, col0:col0 + nb * kk].rearrange(
                            "p (n k) -> p n k", k=kk),
                        axis=mybir.AxisListType.X,
                        op=mybir.AluOpType.add,
                    )
                    if first:
                        inst._wait_ge(s_fence, 16 * nch)
                        first = False

            # ---- head scalars: s = W1@W2, t = b1@W2 + b2 ---------------
            nc.vector.tensor_mul(t_tmp8[:], t_w1[:], t_w2[:])
            nc.vector.tensor_reduce(out=t_uv[:, 0:1], in_=t_tmp8[:],
                                    axis=mybir.AxisListType.X,
                                    op=mybir.AluOpType.add)
            nc.vector.tensor_mul(t_tmp8[:], t_b1[:], t_w2[:])
            nc.vector.tensor_reduce(out=t_uv[:, 1:2], in_=t_tmp8[:],
                                    axis=mybir.AxisListType.X,
                                    op=mybir.AluOpType.add)
            nc.tensor.matmul(out=p_st[:], lhsT=t_ones_col[:], rhs=t_uv[:],
                             start=True, stop=True)
            nc.vector.tensor_copy(t_st_row[:], p_st[:])
            nc.vector.tensor_add(t_st_row[:, 1:2], t_st_row[:, 1:2], t_b2[:])
            nc.tensor.matmul(out=p_bc[:], lhsT=t_ones_row[:], rhs=t_st_row[:],
                             start=True, stop=True)
            nc.vector.tensor_copy(t_st_bc[:], p_bc[:])

            # ---- epilogue: out = s*(dinv*z + dinv^2*state) + t ---------
            nc.scalar.activation(t_dinvp[:], t_dperm[:],
                                 mybir.ActivationFunctionType.Sqrt,
                                 bias=t_bias0[:])
            nc.vector.reciprocal(t_dinvp[:], t_dinvp[:])
            nc.vector.tensor_mul(t_out[:], t_dinvp[:], t_sperm[:])
            nc.vector.tensor_mul(t_out[:], t_out[:], t_dinvp[:])
            nc.vector.tensor_mul(t_z[:], t_z[:], t_dinvp[:])
            nc.vector.tensor_add(t_out[:], t_out[:], t_z[:])
            nc.vector.tensor_mul(
                t_out[:], t_out[:], t_st_bc[:, 0:1].to_broadcast([P, n_slots]))
            nc.vector.tensor_add(
                t_out[:], t_out[:], t_st_bc[:, 1:2].to_broadcast([P, n_slots]))
            nc.sync.dma_start(ap_out[:], t_out[:])

    nc.compile()
    return nc


# ----------------------------------------------------------------------------
# Public entry point
# ----------------------------------------------------------------------------
def kernel(state, edge_index, W1, b1, W2, b2):
    _install_axon_shim()
    from concourse import bass_utils

    state = np.asarray(state, dtype=np.float32)
    W1 = np.asarray(W1, dtype=np.float32)
    b1 = np.asarray(b1, dtype=np.float32)
    W2 = np.asarray(W2, dtype=np.float32)
    b2 = np.asarray(b2, dtype=np.float32)

    prep = host_prep(np.asarray(edge_index))
    deg_all = prep["deg_all"]
    G, ncols, runs, n_slots = prep["G"], prep["ncols"], prep["runs"], prep["n_slots"]

    key = (G, ncols, tuple(runs), n_slots)
    if key not in _cached:
        _cached.clear()
        _cached[key] = _build_kernel(G, ncols, runs, n_slots)
    nc = _cached[key]

    state2d = np.zeros((P, TAB_COLS), dtype=np.float32)
    state2d.reshape(-1)[:N] = state
    deg2d = np.ones((P, TAB_COLS), dtype=np.float32)
    deg2d.reshape(-1)[:N] = deg_all + 1.0
    w1r = W1.reshape(P, H // P)
    w2r = W2.reshape(P, H // P)
    b1r = b1.reshape(P, H // P)
    b2v = b2.reshape(1, 1)

    in_maps = []
    for c in prep["cores"]:
        sn = c["slot_nodes"]
        sperm = np.zeros((P, n_slots), dtype=np.float32)
        dperm = np.ones((P, n_slots), dtype=np.float32)
        v = sn >= 0
        sperm[v] = state[sn[v]]
        dperm[v] = deg_all[sn[v]] + 1.0
        in_maps.append({
            "state2d": state2d, "deg2d": deg2d, "offs": c["packed"],
            "state_perm": sperm, "deg_perm": dperm,
            "w1r": w1r, "w2r": w2r, "b1r": b1r, "b2v": b2v,
        })

    trace = bool(int(os.environ.get("KERNEL_TRACE", "0")))
    res = None
    for attempt in range(3):
        try:
            res = bass_utils.run_bass_kernel_spmd(
                nc, in_maps, list(range(NCORES)), trace=trace)
            break
        except Exception:
            if attempt == 2:
                raise
            import time
            time.sleep(45)   # transient accelerator-state errors recover
    kernel._last_res = res

    out = np.zeros((N, 1), dtype=np.float32)
    for k, c in enumerate(prep["cores"]):
        sn = c["slot_nodes"]
        v = sn >= 0
        out[sn[v], 0] = res.results[k]["outp"][v]
    return out



# revision 21
# speedup vs baseline: 1.0396x; 1.0396x over previous
"""Trainium2 Bass kernel for a scalar-feature GCN critic head.

Math (rank-1 collapse of the reference):
    deg  = bincount(dst) + 1
    dinv = rsqrt(deg)
    y    = state * dinv
    z[d] = sum_{e: dst[e]=d} y[src[e]]
    agg  = dinv * z + dinv^2 * state
    out  = agg * (W1[0] @ W2) + (b1 @ W2 + b2)          # (N, 1)

Sharding: nodes are partitioned across 8 NeuronCores by destination id
(graph/data parallel); edges are bucketed by destination on the host so
every segment-sum stays device-local.  Each core rebuilds the full y
table (1 MB) itself, so no collectives are needed.  On-device, the
per-edge y[src] fetch uses per-partition indirect DMAs (one SWDGE call
per partition row; each descriptor gathers one 4-byte element), and the
per-node segment sum is a fully regular degree-classed padded-CSR
tensor_reduce.  The tiny W1/W2/b1/b2 head collapses to two scalars
computed on-device with two small PE matmuls.
"""

import sys
import os
import types
import numpy as np

sys.path.insert(0, "/opt/trn_rl_repo")

N = 250_000
E = 8_000_000
H = 1024
NCORES = 8
NPC = N // NCORES            # 31250 dst nodes per core
P = 128
NODES_PER_PART = (NPC + P - 1) // P      # 245 node slots per partition
TAB_COLS = 1954              # 128*1954 = 250112 >= N (y-table layout)
NTAB = P * TAB_COLS
DUMMY = NTAB - 1             # pad gather index (y == 0 there)
GATHER_NSUB = 1              # sub-calls per partition row
CH = 4                       # gather chunks per partition row
NREP = 1                     # y-table replicas in DRAM (HBM bank spread)

# partition visited by (engine e, round r): consecutive calls hit distinct
# SDMA engines so all 16 rings stay fed (port swizzle: engine k serves
# partitions {4k..4k+3, 4k+32..} interleaved even/odd across halves)
def _engine_order():
    order = []
    for r in range(8):
        for e in range(16):
            p = 4 * (e // 2) + 64 * (e % 2) + 32 * (r // 4) + (r % 4)
            order.append(p)
    return order


def _chunk_widths(ncols):
    base = ncols // CH
    return [base + (1 if i < ncols % CH else 0) for i in range(CH)]

_cached = {}


def _install_axon_shim():
    """Restore the missing antenv.axon_hooks NTFF hook (profiling only) and
    neutralize artifact upload. Harmless when tracing is off."""
    if "antenv.axon_hooks" not in sys.modules:
        mod = types.ModuleType("antenv.axon_hooks")
        mod._hook = None
        mod.set_axon_ntff_profile_hook = lambda h: setattr(mod, "_hook", h)
        mod.get_axon_ntff_profile_hook = lambda: mod._hook
        sys.modules["antenv.axon_hooks"] = mod
        try:
            import antenv
            antenv.axon_hooks = mod
        except Exception:
            pass
    mod = sys.modules["antenv.axon_hooks"]
    if mod._hook is None:
        try:
            from trn_agent_boot.trn_boot import _ntff_profile_via_ctypes
            mod._hook = _ntff_profile_via_ctypes("/opt/axon/libaxon_pjrt.so")
        except Exception:
            pass
    try:
        from concourse import bass_utils
        bass_utils.upload_artifacts = lambda tmpdir: tmpdir
    except Exception:
        pass


def _ceil_to(x, m):
    return (x + m - 1) // m * m


def _ranges(counts):
    counts = np.asarray(counts, dtype=np.int64)
    total = int(counts.sum())
    if total == 0:
        return np.zeros(0, dtype=np.int64)
    ends = np.cumsum(counts)
    out = np.arange(total, dtype=np.int64)
    out -= np.repeat(ends - counts, counts)
    return out


# ----------------------------------------------------------------------------
# Host-side sharding / bucketing (index-only preprocessing)
# ----------------------------------------------------------------------------
def host_prep(edge_index):
    """Bucket edges by destination core and build, for each core:
      - packed   : [128, 128*ncols] int32 gather-offset tile (one column
                   window per indirect-DMA call, packed partition-fastest)
      - slot_nodes: [128, n_slots] original node id per CSR slot (-1 = pad)
    plus the shared reduce run-list (col0, j0, nbatch, K)."""
    src = np.ascontiguousarray(edge_index[0]).astype(np.int64)
    dst = np.ascontiguousarray(edge_index[1]).astype(np.int64)

    deg_all = np.bincount(dst, minlength=N).astype(np.int64)
    order = np.argsort(dst, kind="stable")
    src_s = src[order]
    rowptr = np.zeros(N + 1, dtype=np.int64)
    np.cumsum(deg_all, out=rowptr[1:])

    # Global degree-sorted round-robin deal: node of rank g goes to
    # (core (g%1024)//128, partition (g%1024)%128, slot g//1024), so the
    # 1024 rows have near-identical degree multisets rank-by-rank and the
    # shared per-slot K padding is ~zero.
    ROWS = NCORES * P
    n_slots = (N + ROWS - 1) // ROWS
    by_deg = np.argsort(-deg_all, kind="stable")
    dealt = np.full(n_slots * ROWS, -1, dtype=np.int64)
    dealt[:N] = by_deg
    dealt = dealt.reshape(n_slots, NCORES, P)            # [j, core, p]
    Kshared = np.full(n_slots, 2, dtype=np.int64)
    Kshared = np.maximum(Kshared, deg_all[by_deg[::ROWS]][:n_slots])
    per_core_nodes = [dealt[:, k, :] for k in range(NCORES)]  # [j, p]

    offs = np.zeros(n_slots + 1, dtype=np.int64)
    np.cumsum(Kshared, out=offs[1:])
    G = _ceil_to(int(offs[-1]), P)
    ncols = G // P

    runs = []
    j = 0
    while j < n_slots:
        j2 = j
        while j2 < n_slots and Kshared[j2] == Kshared[j]:
            j2 += 1
        runs.append((int(offs[j]), j, j2 - j, int(Kshared[j])))
        j = j2

    cores = []
    for k in range(NCORES):
        slot_nodes = per_core_nodes[k]
        offs_logical = np.full((P, G), DUMMY, dtype=np.int64)
        jj, pp = np.nonzero(slot_nodes >= 0)
        nn = slot_nodes[jj, pp]
        dd = deg_all[nn]
        edge_rows = np.repeat(pp, dd)
        tgt_pos = np.repeat(offs[jj], dd) + _ranges(dd)
        src_pos = np.repeat(rowptr[nn], dd) + _ranges(dd)
        offs_logical[edge_rows, tgt_pos] = src_s[src_pos]
        # spread m2s reads across table replicas: consecutive consumed
        # descriptors (partition-fastest) hit different 1 MB DRAM regions
        offs_logical += (np.arange(P)[:, None] % NREP) * NTAB

        # call (p, c) consumes offsets partition-fastest within its window:
        # packed[g%128, p*ncols + w0_c + g//128] = logical[p, P*w0_c + g]
        nws = _chunk_widths(ncols)
        packed = np.empty((P, P * ncols), dtype=np.int32)
        for p in range(P):
            w0 = 0
            for nw in nws:
                packed[:, p * ncols + w0:p * ncols + w0 + nw] = \
                    offs_logical[p, P * w0:P * (w0 + nw)].reshape(nw, P).T
                w0 += nw
        cores.append(dict(packed=packed, slot_nodes=slot_nodes.T))

    return dict(deg_all=deg_all, cores=cores, runs=runs, G=G, ncols=ncols,
                n_slots=n_slots)


# ----------------------------------------------------------------------------
# Device kernel construction
# ----------------------------------------------------------------------------
def _build_kernel(G, ncols, runs, n_slots):
    from concourse import bass, bacc, tile, mybir
    from contextlib import ExitStack

    nc = bacc.Bacc("TRN2", target_bir_lowering=False, debug=False,
                   num_devices=NCORES)
    f32 = mybir.dt.float32

    ap_state = nc.dram_tensor("state2d", (P, TAB_COLS), f32, kind="ExternalInput").ap()
    ap_deg = nc.dram_tensor("deg2d", (P, TAB_COLS), f32, kind="ExternalInput").ap()
    ap_offs = nc.dram_tensor("offs", (P, P * ncols), mybir.dt.int32, kind="ExternalInput").ap()
    ap_sperm = nc.dram_tensor("state_perm", (P, n_slots), f32, kind="ExternalInput").ap()
    ap_dperm = nc.dram_tensor("deg_perm", (P, n_slots), f32, kind="ExternalInput").ap()
    ap_w1 = nc.dram_tensor("w1r", (P, H // P), f32, kind="ExternalInput").ap()
    ap_w2 = nc.dram_tensor("w2r", (P, H // P), f32, kind="ExternalInput").ap()
    ap_b1 = nc.dram_tensor("b1r", (P, H // P), f32, kind="ExternalInput").ap()
    ap_b2 = nc.dram_tensor("b2v", (1, 1), f32, kind="ExternalInput").ap()
    ap_out = nc.dram_tensor("outp", (P, n_slots), f32, kind="ExternalOutput").ap()
    y_dram = nc.dram_tensor("y_scratch", (NREP * NTAB, 1), f32, kind="Internal").ap()

    with tile.TileContext(nc) as tc:
        with ExitStack() as ctx:
            pool = ctx.enter_context(tc.tile_pool(name="sbuf", bufs=1))
            psum = ctx.enter_context(tc.tile_pool(name="psum", bufs=1, space="PSUM"))

            t_state = pool.tile([P, TAB_COLS], f32)
            t_deg = pool.tile([P, TAB_COLS], f32)
            t_dinv = pool.tile([P, TAB_COLS], f32)
            t_y = pool.tile([P, TAB_COLS], f32)
            t_offs = pool.tile([P, P * ncols], mybir.dt.int32)
            t_m = pool.tile([P, G + 1], f32)
            t_fence = pool.tile([P, 1], f32)
            t_dummy = pool.tile([P, 1], mybir.dt.int32)
            t_z = pool.tile([P, n_slots], f32)
            t_sperm = pool.tile([P, n_slots], f32)
            t_dperm = pool.tile([P, n_slots], f32)
            t_dinvp = pool.tile([P, n_slots], f32)
            t_w1 = pool.tile([P, H // P], f32)
            t_w2 = pool.tile([P, H // P], f32)
            t_b1 = pool.tile([P, H // P], f32)
            t_b2 = pool.tile([1, 1], f32)
            t_tmp8 = pool.tile([P, H // P], f32)
            t_uv = pool.tile([P, 2], f32)
            t_ones_col = pool.tile([P, 1], f32)
            t_ones_row = pool.tile([1, P], f32)
            t_st_row = pool.tile([1, 2], f32)
            t_st_bc = pool.tile([P, 2], f32)
            t_bias0 = pool.tile([P, 1], f32)
            t_out = pool.tile([P, n_slots], f32)
            p_st = psum.tile([1, 2], f32, space="PSUM")
            p_bc = psum.tile([P, 2], f32, space="PSUM")

            s_y = nc.alloc_semaphore("s_y")
            s_fence = nc.alloc_semaphore("s_fence")

            # ---- loads -------------------------------------------------
            nc.sync.dma_start(t_state[:], ap_state[:])
            nc.sync.dma_start(t_deg[:], ap_deg[:])
            nc.sync.dma_start(t_offs[:], ap_offs[:])
            nc.sync.dma_start(t_sperm[:], ap_sperm[:])
            nc.sync.dma_start(t_dperm[:], ap_dperm[:])
            nc.sync.dma_start(t_w1[:], ap_w1[:])
            nc.sync.dma_start(t_w2[:], ap_w2[:])
            nc.sync.dma_start(t_b1[:], ap_b1[:])
            nc.sync.dma_start(t_b2[:], ap_b2[:])
            nc.gpsimd.memset(t_fence[:], 1.0)
            nc.gpsimd.memset(t_bias0[:], 0.0)
            nc.gpsimd.memset(t_ones_col[:], 1.0)
            nc.gpsimd.memset(t_ones_row[:], 1.0)

            # ---- y table: y = state * rsqrt(deg) ----------------------
            nc.scalar.activation(t_dinv[:], t_deg[:],
                                 mybir.ActivationFunctionType.Sqrt,
                                 bias=t_bias0[:])
            nc.vector.reciprocal(t_dinv[:], t_dinv[:])
            nc.vector.tensor_mul(t_y[:], t_state[:], t_dinv[:])
            for r in range(NREP):
                nc.gpsimd.dma_start(
                    y_dram[r * NTAB:(r + 1) * NTAB].rearrange(
                        "(p c) d -> p (c d)", p=P), t_y[:]
                ).then_inc(s_y, 16)

            # ensure offsets resident before descriptor generation
            nc.gpsimd.tensor_copy(t_dummy[:], t_offs[:, :1])

            # ---- gather: per-partition indirect DMAs, engine-interleaved --
            nch = 1
            nws = _chunk_widths(ncols)
            order = _engine_order()
            with nc.named_scope("gather"):
                first = True
                w0 = 0
                for nw in nws:
                    if nw == 0:
                        continue
                    for p in order:
                        inst = nc.gpsimd.indirect_dma_start(
                            out=t_m[p:p + 1, P * w0:P * (w0 + nw)].rearrange(
                                "p (g d) -> p g d", d=1),
                            out_offset=None,
                            in_=y_dram[:],
                            in_offset=bass.IndirectOffsetOnAxis(
                                ap=t_offs[:, p * ncols + w0:
                                          p * ncols + w0 + nw], axis=0),
                        )
                        if first:
                            inst._wait_ge(s_y, 16 * NREP)
                            first = False
                    w0 += nw
                # fence: same SWDGE ring -> FIFO drain; completion sem below
                nc.gpsimd.dma_start(t_m[:, G:G + 1], t_fence[:]) \
                    .then_inc(s_fence, 16)

            # ---- degree-classed segment reduce ------------------------
            with nc.named_scope("reduce"):
                first = True
                for (col0, j0, nb, kk) in runs:
                    inst = nc.vector.tensor_reduce(
                        out=t_z[:, j0:j0 + nb],
                        in_=t_m[:, col0:col0 + nb * kk].rearrange(
                            "p (n k) -> p n k", k=kk),
                        axis=mybir.AxisListType.X,
                        op=mybir.AluOpType.add,
                    )
                    if first:
                        inst._wait_ge(s_fence, 16 * nch)
                        first = False

            # ---- head scalars: s = W1@W2, t = b1@W2 + b2 ---------------
            nc.vector.tensor_mul(t_tmp8[:], t_w1[:], t_w2[:])
            nc.vector.tensor_reduce(out=t_uv[:, 0:1], in_=t_tmp8[:],
                                    axis=mybir.AxisListType.X,
                                    op=mybir.AluOpType.add)
            nc.vector.tensor_mul(t_tmp8[:], t_b1[:], t_w2[:])
            nc.vector.tensor_reduce(out=t_uv[:, 1:2], in_=t_tmp8[:],
                                    axis=mybir.AxisListType.X,
                                    op=mybir.AluOpType.add)
            nc.tensor.matmul(out=p_st[:], lhsT=t_ones_col[:], rhs=t_uv[:],
                             start=True, stop=True)
            nc.vector.tensor_copy(t_st_row[:], p_st[:])
            nc.vector.tensor_add(t_st_row[:, 1:2], t_st_row[:, 1:2], t_b2[:])
            nc.tensor.matmul(out=p_bc[:], lhsT=t_ones_row[:], rhs=t_st_row[:],
                             start=True, stop=True)
            nc.vector.tensor_copy(t_st_bc[:], p_bc[:])

            # ---- epilogue: out = s*(dinv*z + dinv^2*state) + t ---------
            nc.scalar.activation(t_dinvp[:], t_dperm[:],
                                 mybir.ActivationFunctionType.Sqrt,
                                 bias=t_bias0[:])
            nc.vector.reciprocal(t_dinvp[:], t_dinvp[:])
            nc.vector.tensor_mul(t_out[:], t_dinvp[:], t_sperm[:])
            nc.vector.tensor_mul(t_out[:], t_out[:], t_dinvp[:])
            nc.vector.tensor_mul(t_z[:], t_z[:], t_dinvp[:])
            nc.vector.tensor_add(t_out[:], t_out[:], t_z[:])
            nc.vector.tensor_mul(
                t_out[:], t_out[:], t_st_bc[:, 0:1].to_broadcast([P, n_slots]))
            nc.vector.tensor_add(
                t_out[:], t_out[:], t_st_bc[:, 1:2].to_broadcast([P, n_slots]))
            nc.sync.dma_start(ap_out[:], t_out[:])

    nc.compile()
    return nc


# ----------------------------------------------------------------------------
# Public entry point
# ----------------------------------------------------------------------------
def kernel(state, edge_index, W1, b1, W2, b2):
    _install_axon_shim()
    from concourse import bass_utils

    state = np.asarray(state, dtype=np.float32)
    W1 = np.asarray(W1, dtype=np.float32)
    b1 = np.asarray(b1, dtype=np.float32)
    W2 = np.asarray(W2, dtype=np.float32)
    b2 = np.asarray(b2, dtype=np.float32)

    prep = host_prep(np.asarray(edge_index))
    deg_all = prep["deg_all"]
    G, ncols, runs, n_slots = prep["G"], prep["ncols"], prep["runs"], prep["n_slots"]

    key = (G, ncols, tuple(runs), n_slots)
    if key not in _cached:
        _cached.clear()
        _cached[key] = _build_kernel(G, ncols, runs, n_slots)
    nc = _cached[key]

    state2d = np.zeros((P, TAB_COLS), dtype=np.float32)
    state2d.reshape(-1)[:N] = state
    deg2d = np.ones((P, TAB_COLS), dtype=np.float32)
    deg2d.reshape(-1)[:N] = deg_all + 1.0
    w1r = W1.reshape(P, H // P)
    w2r = W2.reshape(P, H // P)
    b1r = b1.reshape(P, H // P)
    b2v = b2.reshape(1, 1)

    in_maps = []
    for c in prep["cores"]:
        sn = c["slot_nodes"]
        sperm = np.zeros((P, n_slots), dtype=np.float32)
        dperm = np.ones((P, n_slots), dtype=np.float32)
        v = sn >= 0
        sperm[v] = state[sn[v]]
        dperm[v] = deg_all[sn[v]] + 1.0
        in_maps.append({
            "state2d": state2d, "deg2d": deg2d, "offs": c["packed"],
            "state_perm": sperm, "deg_perm": dperm,
            "w1r": w1r, "w2r": w2r, "b1r": b1r, "b2v": b2v,
        })

    trace = bool(int(os.environ.get("KERNEL_TRACE", "0")))
    res = None
    for attempt in range(3):
        try:
            res = bass_utils.run_bass_kernel_spmd(
                nc, in_maps, list(range(NCORES)), trace=trace)
            break
        except Exception:
            if attempt == 2:
                raise
            import time
            time.sleep(45)   # transient accelerator-state errors recover
    kernel._last_res = res

    out = np.zeros((N, 1), dtype=np.float32)
    for k, c in enumerate(prep["cores"]):
        sn = c["slot_nodes"]
        v = sn >= 0
        out[sn[v], 0] = res.results[k]["outp"][v]
    return out

